# revision 18
# baseline (speedup 1.0000x reference)
"""Trainium2 Bass kernel for a Conv-TasNet-style decoder (mask * wave ->
overlap_and_add -> trim).

Reference computation (per batch element b):
    A[c, d, t] = x[b, c, d, t] * x_wave[b, d, t]          (broadcast over c)
    frames     = A transposed to [c, t, d]  (frame length D=16, hop 8)
    unsliced   = overlap_and_add(frames, 8)               # [c, (T+1)*8]
    y          = unsliced[:, pad_left : -pad_right]

With hop=8 and D=16, overlap_and_add decomposes into two interleaved
streams, and for the middle region (everything when pads are 8):

    y[c][8s + r] = x[c, r, s+1]*w[r, s+1] + x[c, r+8, s]*w[r+8, s]

i.e. purely elementwise over s plus an 8-way interleave.  The device
kernel computes this on a [128 partitions x 8000] grid (partition p
owns frames [p*1000, (p+1)*1000)); the +1 frame shift is baked into
the DMA-load access patterns (flat-offset views) and the (s, r)
interleave into the final add's write access pattern.  The last 8
elements of the [2, 1024000] padded device output are garbage (frame
index T) and are trimmed on the host.

Schedule: chunks of the 1000-frame-per-partition block are processed
k-outer / speaker-inner so each W chunk is loaded once (interleaved
with the x strea in exact compute order -- never a bulk W load that
starves the vector engine) and consumed by both speakers back to
back.  Low-side loads ride the SP HWDGE queue, high-side the ACT
queue, stores the Pool SWDGE queue; the three queues share the 16 DMA
engines and sustain ~400 GB/s aggregate.  x tiles are quadruple-
buffered so the loads stream at full rate ahead of compute.

Sharding: pure data parallel -- core b computes batch element b (B=8
matches the 8 NeuronCores); no cross-core communication.
"""

import numpy as np

_B, _C, _D, _T = 8, 2, 16, 128000
_HOP = 8
_S = _T * _HOP            # padded per-speaker device output length (1024000)
_MID = _S - _HOP          # valid middle length (1023992)
_P = 128                  # SBUF partitions
_JB = _T // _P            # frames per partition block (1000)

# Chunks tiling the 1000-frame block: small first chunk (fast pipeline
# ramp: first compute only needs 1 MB of loads) and small last chunk
# (short serial tail after the final load).  All DMA runs >= 512 B.
_CHUNKS = [(0, 128), (128, 250), (378, 250), (628, 244), (872, 128)]
_FCMAX = 256

_cached = None            # (nc, run_bass_kernel_spmd)


def _build():
    """Build the Bass module (one NeuronCore's program). Cached."""
    global _cached
    if _cached is not None:
        return _cached

    import concourse.bacc as bacc
    import concourse.mybir as mybir
    import concourse.tile as tile
    from concourse.bass_utils import run_bass_kernel_spmd

    f32 = mybir.dt.float32
    T, P = _T, _P

    nc = bacc.Bacc(debug=False)
    x = nc.declare_dram_parameter("x", [_C, _D, T], f32, isOutput=False)
    w = nc.declare_dram_parameter("x_wave", [_D, T], f32, isOutput=False)
    y = nc.declare_dram_parameter("y_pad", [_C, _S], f32, isOutput=True)

    # Flat 1-D views let us bake the +1-frame shift into the AP offset
    # (a shifted [r, s] view crosses row boundaries, which plain
    # slice-then-rearrange cannot express).
    xf = x[:].rearrange("c d t -> (c d t)")
    wf = w[:].rearrange("d t -> (d t)")
    yf = y[:].rearrange("c n -> (c n)")

    def rpj(flat, start):
        # [p, r, j] view: element = flat[start + r*T + p*_JB + j]
        return flat[start : start + 8 * T].rearrange("(r p j) -> p r j", r=8, p=P)

    wl_full = rpj(wf, 1)          # w[r, s+1]
    wh_full = rpj(wf, 8 * T)      # w[r+8, s]

    with tile.TileContext(nc) as tc:
        with (
            tc.tile_pool(name="wpool", bufs=2) as wpool,
            tc.tile_pool(name="xpool", bufs=6) as xpool,
            tc.tile_pool(name="ppool", bufs=2) as ppool,
            tc.tile_pool(name="zpool", bufs=4) as zpool,
        ):
            pending_store = None
            for j0, fc in _CHUNKS:
                # Per-chunk W tiles, shared by both speakers then freed.
                wlt = wpool.tile([P, 8, _FCMAX], f32, tag="wl", name="wlt")[:, :, :fc]
                nc.sync.dma_start(out=wlt[:], in_=wl_full[:, :, j0 : j0 + fc])
                wht = wpool.tile([P, 8, _FCMAX], f32, tag="wh", name="wht")[:, :, :fc]
                nc.scalar.dma_start(out=wht[:], in_=wh_full[:, :, j0 : j0 + fc])

                for c in range(_C):
                    base = c * _D * T
                    xl_full = rpj(xf, base + 1)      # x[c, r, s+1]
                    xh_full = rpj(xf, base + 8 * T)  # x[c, r+8, s]
                    y_c = yf[c * _S : (c + 1) * _S].rearrange(
                        "(p q) -> p q", p=P
                    )

                    xlt = xpool.tile([P, 8, _FCMAX], f32, tag="xl", name="xlt")[
                        :, :, :fc
                    ]
                    nc.sync.dma_start(out=xlt[:], in_=xl_full[:, :, j0 : j0 + fc])
                    xht = xpool.tile([P, 8, _FCMAX], f32, tag="xh", name="xht")[
                        :, :, :fc
                    ]
                    nc.scalar.dma_start(out=xht[:], in_=xh_full[:, :, j0 : j0 + fc])

                    # Products on DVE with contiguous APs (full rate);
                    # the final add reads contiguously and scatters the
                    # (r, j) -> 8j + r interleave into its write AP.
                    yt = ppool.tile([P, 8 * _FCMAX], f32, tag="yt", name="yt")[
                        :, : 8 * fc
                    ]
                    tt = ppool.tile([P, 8 * _FCMAX], f32, tag="tt", name="tt")[
                        :, : 8 * fc
                    ]
                    zt = zpool.tile([P, 8 * _FCMAX], f32, tag="zt", name="zt")[
                        :, : 8 * fc
                    ]
                    # High-side product on the (otherwise idle) Pool
                    # engine: under sustained load the DVE clock
                    # throttles ~20% and DVE becomes the tail pacer;
                    # offloading one of the three ops cuts its serial
                    # time ~30%.  Pool issues this mul BEFORE the
                    # previous iteration's store (see pending_store) so
                    # the store's semaphore wait cannot stall it.
                    nc.gpsimd.tensor_mul(tt[:], xht[:], wht[:])
                    nc.vector.tensor_mul(yt[:], xlt[:], wlt[:])
                    nc.vector.tensor_add(
                        zt[:],
                        yt.rearrange("p (r j) -> p j r", r=8),
                        tt.rearrange("p (r j) -> p j r", r=8),
                    )
                    # Stores ride the SWDGE (gpsimd) queue - a third DMA
                    # queue with few, large descriptors (8 KB/partition).
                    # The last two stores take the by-then-idle HWDGE
                    # queues instead (lower fixed overhead, parallel
                    # drain with the SWDGE backlog).
                    if (j0, c) == (_CHUNKS[-1][0], 0):
                        store_eng = nc.sync
                    elif (j0, c) == (_CHUNKS[-1][0], 1):
                        store_eng = nc.scalar
                    else:
                        store_eng = nc.gpsimd
                    if pending_store is not None:
                        ps_eng, ps_out, ps_zt = pending_store
                        ps_eng.dma_start(out=ps_out, in_=ps_zt[:])
                    pending_store = (
                        store_eng,
                        y_c[:, 8 * j0 : 8 * (j0 + fc)],
                        zt,
                    )

            if pending_store is not None:
                ps_eng, ps_out, ps_zt = pending_store
                ps_eng.dma_start(out=ps_out, in_=ps_zt[:])

    nc.compile()  # legalize sync waits (>=1 wait/inst split into events)

    _cached = (nc, run_bass_kernel_spmd)
    return _cached


def _run_device(x, w, trace=False):
    nc, run_bass_kernel_spmd = _build()
    in_maps = [
        {"x": np.ascontiguousarray(x[b]), "x_wave": np.ascontiguousarray(w[b])}
        for b in range(_B)
    ]
    res = run_bass_kernel_spmd(nc, in_maps, core_ids=list(range(_B)), trace=trace)
    mid = np.stack([r["y_pad"][:, :_MID] for r in res.results])
    return mid, res


def kernel(x, x_wave, pad_left=8, pad_right=8, _trace=False, _return_res=False):
    x = np.asarray(x, dtype=np.float32)
    w = np.asarray(x_wave, dtype=np.float32)
    pl, pr = int(pad_left), int(pad_right)
    assert x.shape == (_B, _C, _D, _T) and w.shape == (_B, _D, _T)

    mid, res = _run_device(x, w, trace=_trace)

    if pl == 8 and pr == 8:
        out = mid
    else:
        # General trim: reconstruct the 8 leading / 8 trailing elements
        # of the unsliced overlap-add on the host (they only involve the
        # first/last frame) and slice.
        front = x[:, :, 0:8, 0] * w[:, None, 0:8, 0]        # unsliced[0:8]
        back = x[:, :, 8:16, -1] * w[:, None, 8:16, -1]     # unsliced[-8:]
        full = np.concatenate([front, mid, back], axis=-1)  # [B, C, (T+1)*8]
        end = full.shape[-1] - pr
        out = np.ascontiguousarray(full[:, :, pl:end])

    if _return_res:
        return out, res
    return out


# revision 19
# speedup vs baseline: 1.2647x; 1.2647x over previous
"""Trainium2 Bass kernel for a Conv-TasNet-style decoder (mask * wave ->
overlap_and_add -> trim).

Reference computation (per batch element b):
    A[c, d, t] = x[b, c, d, t] * x_wave[b, d, t]          (broadcast over c)
    frames     = A transposed to [c, t, d]  (frame length D=16, hop 8)
    unsliced   = overlap_and_add(frames, 8)               # [c, (T+1)*8]
    y          = unsliced[:, pad_left : -pad_right]

With hop=8 and D=16, overlap_and_add decomposes into two interleaved
streams, and for the middle region (everything when pads are 8):

    y[c][8s + r] = x[c, r, s+1]*w[r, s+1] + x[c, r+8, s]*w[r+8, s]

i.e. purely elementwise over s plus an 8-way interleave.  The device
kernel computes this on a [128 partitions x 8000] grid (partition p
owns frames [p*1000, (p+1)*1000)); the +1 frame shift is baked into
the DMA-load access patterns (flat-offset views) and the (s, r)
interleave into the final add's write access pattern.  The last 8
elements of the [2, 1024000] padded device output are garbage (frame
index T) and are trimmed on the host.

Schedule: chunks of the 1000-frame-per-partition block are processed
k-outer / speaker-inner so each W chunk is loaded once (interleaved
with the x strea in exact compute order -- never a bulk W load that
starves the vector engine) and consumed by both speakers back to
back.  Low-side loads ride the SP HWDGE queue, high-side the ACT
queue, stores the Pool SWDGE queue; the three queues share the 16 DMA
engines and sustain ~400 GB/s aggregate.  x tiles are quadruple-
buffered so the loads stream at full rate ahead of compute.

Sharding: pure data parallel -- core b computes batch element b (B=8
matches the 8 NeuronCores); no cross-core communication.
"""

import numpy as np

_B, _C, _D, _T = 8, 2, 16, 128000
_HOP = 8
_S = _T * _HOP            # padded per-speaker device output length (1024000)
_MID = _S - _HOP          # valid middle length (1023992)
_P = 128                  # SBUF partitions
_JB = _T // _P            # frames per partition block (1000)

# Chunks tiling the 1000-frame block: small first chunk (fast pipeline
# ramp: first compute only needs 1 MB of loads) and small last chunk
# (short serial tail after the final load).  All DMA runs >= 512 B.
_CHUNKS = [(0, 128), (128, 250), (378, 250), (628, 244), (872, 128)]
_FCMAX = 256

_cached = None            # (nc, run_bass_kernel_spmd)


def _build():
    """Build the Bass module (one NeuronCore's program). Cached."""
    global _cached
    if _cached is not None:
        return _cached

    import concourse.bacc as bacc
    import concourse.mybir as mybir
    import concourse.tile as tile
    from concourse.bass_utils import run_bass_kernel_spmd

    f32 = mybir.dt.float32
    T, P = _T, _P

    nc = bacc.Bacc(debug=False)
    x = nc.declare_dram_parameter("x", [_C, _D, T], f32, isOutput=False)
    w = nc.declare_dram_parameter("x_wave", [_D, T], f32, isOutput=False)
    y = nc.declare_dram_parameter("y_pad", [_C, _S], f32, isOutput=True)

    # Flat 1-D views let us bake the +1-frame shift into the AP offset
    # (a shifted [r, s] view crosses row boundaries, which plain
    # slice-then-rearrange cannot express).
    xf = x[:].rearrange("c d t -> (c d t)")
    wf = w[:].rearrange("d t -> (d t)")
    yf = y[:].rearrange("c n -> (c n)")

    def rpj(flat, start):
        # [p, r, j] view: element = flat[start + r*T + p*_JB + j]
        return flat[start : start + 8 * T].rearrange("(r p j) -> p r j", r=8, p=P)

    wl_full = rpj(wf, 1)          # w[r, s+1]
    wh_full = rpj(wf, 8 * T)      # w[r+8, s]

    with tile.TileContext(nc) as tc:
        with (
            tc.tile_pool(name="wpool", bufs=2) as wpool,
            tc.tile_pool(name="xpool", bufs=4) as xpool,
            tc.tile_pool(name="ppool", bufs=2) as ppool,
            tc.tile_pool(name="zpool", bufs=4) as zpool,
        ):
            pending_store = None
            for j0, fc in _CHUNKS:
                # Per-chunk W tiles, shared by both speakers then freed.
                wlt = wpool.tile([P, 8, _FCMAX], f32, tag="wl", name="wlt")[:, :, :fc]
                nc.sync.dma_start(out=wlt[:], in_=wl_full[:, :, j0 : j0 + fc])
                wht = wpool.tile([P, 8, _FCMAX], f32, tag="wh", name="wht")[:, :, :fc]
                nc.scalar.dma_start(out=wht[:], in_=wh_full[:, :, j0 : j0 + fc])

                for c in range(_C):
                    base = c * _D * T
                    xl_full = rpj(xf, base + 1)      # x[c, r, s+1]
                    xh_full = rpj(xf, base + 8 * T)  # x[c, r+8, s]
                    y_c = yf[c * _S : (c + 1) * _S].rearrange(
                        "(p q) -> p q", p=P
                    )

                    xlt = xpool.tile([P, 8, _FCMAX], f32, tag="xl", name="xlt")[
                        :, :, :fc
                    ]
                    nc.sync.dma_start(out=xlt[:], in_=xl_full[:, :, j0 : j0 + fc])
                    xht = xpool.tile([P, 8, _FCMAX], f32, tag="xh", name="xht")[
                        :, :, :fc
                    ]
                    nc.scalar.dma_start(out=xht[:], in_=xh_full[:, :, j0 : j0 + fc])

                    # Products on DVE with contiguous APs (full rate);
                    # the final add reads contiguously and scatters the
                    # (r, j) -> 8j + r interleave into its write AP.
                    yt = ppool.tile([P, 8 * _FCMAX], f32, tag="yt", name="yt")[
                        :, : 8 * fc
                    ]
                    tt = ppool.tile([P, 8 * _FCMAX], f32, tag="tt", name="tt")[
                        :, : 8 * fc
                    ]
                    zt = zpool.tile([P, 8 * _FCMAX], f32, tag="zt", name="zt")[
                        :, : 8 * fc
                    ]
                    # High-side product on the (otherwise idle) Pool
                    # engine: under sustained load the DVE clock
                    # throttles ~20% and DVE becomes the tail pacer;
                    # offloading one of the three ops cuts its serial
                    # time ~30%.  Pool issues this mul BEFORE the
                    # previous iteration's store (see pending_store) so
                    # the store's semaphore wait cannot stall it.
                    nc.gpsimd.tensor_mul(tt[:], xht[:], wht[:])
                    nc.vector.tensor_mul(yt[:], xlt[:], wlt[:])
                    nc.vector.tensor_add(
                        zt[:],
                        yt.rearrange("p (r j) -> p j r", r=8),
                        tt.rearrange("p (r j) -> p j r", r=8),
                    )
                    # Stores ride the SWDGE (gpsimd) queue - a third DMA
                    # queue with few, large descriptors (8 KB/partition).
                    # The last two stores take the by-then-idle HWDGE
                    # queues instead (lower fixed overhead, parallel
                    # drain with the SWDGE backlog).
                    if (j0, c) == (_CHUNKS[-1][0], 0):
                        store_eng = nc.sync
                    elif (j0, c) == (_CHUNKS[-1][0], 1):
                        store_eng = nc.scalar
                    else:
                        store_eng = nc.gpsimd
                    if pending_store is not None:
                        ps_eng, ps_out, ps_zt = pending_store
                        ps_eng.dma_start(out=ps_out, in_=ps_zt[:])
                    pending_store = (
                        store_eng,
                        y_c[:, 8 * j0 : 8 * (j0 + fc)],
                        zt,
                    )

            if pending_store is not None:
                ps_eng, ps_out, ps_zt = pending_store
                ps_eng.dma_start(out=ps_out, in_=ps_zt[:])

    nc.compile()  # legalize sync waits (>=1 wait/inst split into events)

    _cached = (nc, run_bass_kernel_spmd)
    return _cached


def _run_device(x, w, trace=False):
    nc, run_bass_kernel_spmd = _build()
    in_maps = [
        {"x": np.ascontiguousarray(x[b]), "x_wave": np.ascontiguousarray(w[b])}
        for b in range(_B)
    ]
    res = run_bass_kernel_spmd(nc, in_maps, core_ids=list(range(_B)), trace=trace)
    mid = np.stack([r["y_pad"][:, :_MID] for r in res.results])
    return mid, res


def kernel(x, x_wave, pad_left=8, pad_right=8, _trace=False, _return_res=False):
    x = np.asarray(x, dtype=np.float32)
    w = np.asarray(x_wave, dtype=np.float32)
    pl, pr = int(pad_left), int(pad_right)
    assert x.shape == (_B, _C, _D, _T) and w.shape == (_B, _D, _T)

    mid, res = _run_device(x, w, trace=_trace)

    if pl == 8 and pr == 8:
        out = mid
    else:
        # General trim: reconstruct the 8 leading / 8 trailing elements
        # of the unsliced overlap-add on the host (they only involve the
        # first/last frame) and slice.
        front = x[:, :, 0:8, 0] * w[:, None, 0:8, 0]        # unsliced[0:8]
        back = x[:, :, 8:16, -1] * w[:, None, 8:16, -1]     # unsliced[-8:]
        full = np.concatenate([front, mid, back], axis=-1)  # [B, C, (T+1)*8]
        end = full.shape[-1] - pr
        out = np.ascontiguousarray(full[:, :, pl:end])

    if _return_res:
        return out, res
    return out
